# revision 44
# baseline (speedup 1.0000x reference)
"""fp16-output MoE routing kernel (tanh(feat @ W_e + b_e) routed by bus_type).

The correctness gate is rel_err < 2e-2 on fixed-seed inputs. ScalarE tanh
is the primary bottleneck (1 elem/cycle/partition @ 1.2GHz over 125,952
cols/core = 105us + per-ACTIVATE overhead), so part of the tanh work is
offloaded to the DVE as a clamped degree-5 odd minimax polynomial
(end-to-end max err ~1.5e-2, deterministic). DMA is the secondary wall
(output 32.2MB/core at ~380GB/s aggregate).

Data flow per 512-token pack:
- Host pre-masks inputs: 9 f16 slot values/token (bus_type==e+1)*[f0,f1,1]
  in device layout, so the on-device "build" is ONE DVE tensor_copy per
  supertile running in the 4x perf mode (tensor_scalar/tensor_copy support
  4x on fp16 SBUF; scalar_tensor_tensor has NO fast mode — the old
  mask-on-device build was 6x more DVE time).
- P[p, 32g+s] slot matrix (single persistent tile): slots 9..31 zeroed
  once via 4x copies from a zeroed dummy tile (NOT GpSimd memset: GpSimd
  shares an SBUF port with the DVE and a concurrent big memset slows DVE
  4x copies ~20x; its completion sem also lands in coalesced DVE waits).
- DVE stream-transpose (32x32 blocks) per 8 packs -> X32.
- matmul(out, lhsT=X32 slice, rhs=Wbig) with Wbig block-diagonal over
  A-blocks: out[32B+j, A*128+d] = z(token) in PSUM fp32 (TRN2 matmul
  cannot write fp16 PSUM).
- ACT tanh FD 2048 -> fp16 SBUF ob tile; for offloaded groups (gi 1,3,5
  of middle supertiles) ACT does packs 1-3 (FD 1536, which keeps the PSUM
  tile turnover on the normal cadence and stays above the ~1.5us/group PE
  refill floor) while the DVE converts pack 0 (fp32 PSUM -> fp16 SBUF,
  1x) in the same slot and runs the 6-op poly piece one slot later:
  t=s*s (TT 2x); u=C5A*t+C5B (TS 4x); h=u*t; g=h+C5C; y=g*s;
  clamp(+-CLAMP) into ob. The pack-0 output DMA is deferred two slots.
- Output [128, npk*512] per supertile; host permutation keeps HBM lines
  contiguous and the unpack a pure reshape.

Scheduling notes (hard-won):
- Input loads are issued from the Scalar SEQ (667ns issue slots hide
  between TANH dispatches), one merged DMA per supertile, staggered 2
  supertiles ahead; supertile 0 is split 8+24 packs so the first build
  only waits on the 74KB head. The Sync queue stays output-only; the last
  2 output blocks drain via the then-idle Scalar SEQ.
- The Tile scheduler coalesces upcoming DVE waits into one
  EVENT_SEMAPHORE (window spans ~a dozen instructions), so each
  supertile's build is emitted immediately before its first transpose:
  emitting it earlier stalls the whole early DVE batch on that
  supertile's input DMA.
- PE warmup matmuls (2 per group over the first 3 groups, into the
  group's own mm tile just before the real matmuls overwrite it)
  un-throttle the HAM clock gate; ACT gaps let it re-throttle, which
  shows up as slow MATMULs.
"""

import sys
from contextlib import ExitStack

import numpy as np

sys.path.insert(0, "/opt/trn_rl_repo")

import concourse.bacc as bacc  # noqa: E402
import concourse.mybir as mybir  # noqa: E402
import concourse.tile as tile  # noqa: E402
from concourse.bass_utils import run_bass_kernel_spmd  # noqa: E402

FP = mybir.dt.float32
F16 = mybir.dt.float16
D = 128
PACK = 512
SUPERS = [16384] * 7 + [11264]
NPKS = [s // PACK for s in SUPERS]
N_CORES = 8
PER_CORE = sum(SUPERS)

_NC_CACHE = {}


# Clamped odd minimax polynomial for tanh on the DVE-offloaded packs:
# y = clip(x*(C5A*x^4 + C5B*x^2 + C5C), +-CLAMP). LP-fit over z range
# [-4.7, 4.7]; poly error 0.0140, ~0.0155 end to end with the fp16 chain
# (gate is 2e-2, deterministic inputs).
C5A = 0.019074705123782443
C5B = -0.1933162963179165
C5C = 0.9452190991740973
CLAMP = 1.0 - 0.01273


def _body(ctx, tc, out, inc, wbig):
    nc = tc.nc
    mult = mybir.AluOpType.mult
    add = mybir.AluOpType.add
    amin = mybir.AluOpType.min
    amax = mybir.AluOpType.max

    const_pool = ctx.enter_context(tc.tile_pool(name="const", bufs=1))
    # ONE persistent slot matrix: builds are cheap 4x copies now, so super
    # si+1's build simply runs after super si's last transpose (in-order
    # DVE). Slots 9..31 are zeroed once and never written again (their Wbig
    # rows are zero, but NaN garbage would still poison the accumulation).
    # All zeroing stays on the DVE: GpSimd memsets contend with DVE for the
    # shared SBUF port (a concurrent 4x DVE copy was observed to crawl
    # ~20x) and their completion semaphore stalls the first builds. The
    # P tail is zeroed by 4x tensor_copies from the zeroed dummy tile,
    # paced through the first group slots below.
    P_tile = const_pool.tile([128, 32 * 128], F16, name="Pbuf")
    # Zeroed dummy operand for PE warmup (and zero-source for P), then the
    # first 4 packs of P.
    dumw = const_pool.tile([128, 512], F16)
    nc.vector.memset(dumw[:], 0.0)
    nc.vector.memset(P_tile[:, :4 * 128], 0.0)

    in_pool = ctx.enter_context(tc.tile_pool(name="inp", bufs=1))
    x_pool = ctx.enter_context(tc.tile_pool(name="x32", bufs=3))
    mm_pool = ctx.enter_context(tc.tile_pool(name="mm", bufs=2, space="PSUM"))
    ob_pool = ctx.enter_context(tc.tile_pool(name="ob", bufs=10))

    # Per-supertile input tiles: the 9 pre-masked slot values per token (36
    # f16 cols/pack), one dma_start per supertile with 2.25KB-contiguous
    # partition rows. All input loads are issued from the Scalar SEQ: it
    # only dispatches one TANH every ~1.85us, so the ~0.7us issue slots
    # hide there, keeping the Sync queue output-only.
    in_tiles = []
    icol = 0
    for si, npk in enumerate(NPKS):
        inT = in_pool.tile([128, npk * 36], F16, name=f"inT{si}")
        in_tiles.append((inT, icol))
        icol += npk * 36

    def load_inputs(si):
        inT, ic = in_tiles[si]
        npk = NPKS[si]
        nc.scalar.dma_start(inT[:], inc[:, ic:ic + npk * 36])

    # Supertile 0's load goes out first (before the ACT table load), split
    # so the first 8 packs (74KB) land ~2.5us before the rest: the first
    # build is the critical path to the first TANH. Supertile 1 is loaded
    # from inside the loop.
    inT0, _ = in_tiles[0]
    nc.scalar.dma_start(inT0[:, :8 * 36], inc[:, :8 * 36])
    nc.scalar.dma_start(inT0[:, 8 * 36:], inc[:, 8 * 36:NPKS[0] * 36])
    wbig_sb = const_pool.tile([128, 512], F16)
    nc.sync.dma_start(wbig_sb[:], wbig)

    # PE warmup (HAM un-throttle needs ~3.4us of busy) runs on the zeroed
    # dummy operand. Warmups write into the first groups' own mm tiles just
    # before the real matmuls overwrite them (start=True resets PSUM), so
    # no separate PSUM tile is needed and nothing queues behind them.
    def warm(mm, w):
        nc.tensor.matmul(mm[:, (w % 4) * 512:(w % 4 + 1) * 512],
                         dumw[:, :128], dumw[:], start=True, stop=True)

    # Builds run on the DVE, split into chunks small enough to fit the
    # per-group slack between stream-transposes (ACT pace ~1.85us/group,
    # transposes ~0.6us/group).
    def build(si, lo, hi):
        inT, _ = in_tiles[si]
        P4 = P_tile[:, lo * 128:hi * 128].rearrange(
            "p (pt g s) -> p pt g s", g=4, s=32)
        in9 = inT[:, lo * 36:hi * 36].rearrange(
            "p (pt g s) -> p pt g s", g=4, s=9)
        nc.vector.tensor_copy(P4[:, :, :, 0:9], in9)

    build(0, 0, 4)
    # One-time zero fill of P packs 4..32 (4x copies of the zeroed dummy
    # into all 32 slots) ordered BEFORE the build of the same pack range;
    # paced one job list per group slot during super 0's ramp. Slots 9..31
    # are never written again; later builds only rewrite slots 0..8.
    def pzero(lo, hi):
        for c in range(lo * 128, hi * 128, 512):
            nc.vector.tensor_copy(P_tile[:, c:c + 512], dumw[:])

    pzero(4, 8)
    pend = [
        [lambda: build(0, 4, 8)],
        [lambda: pzero(8, 16), lambda: build(0, 8, 16)],
        [lambda: pzero(16, 24), lambda: build(0, 16, 24)],
        [lambda: pzero(24, 32), lambda: build(0, 24, 32)],
    ]

    # Transpose spans per supertile: 8-pack transposes halve the DVE
    # per-op overhead; supertile 0 starts with two 4-pack ones so the first
    # TANH isn't gated on an 8-pack build+transpose.
    def tr_spans(si, npk):
        if si == 0:
            return [4, 4] + [8] * ((npk - 8) // 8)
        spans = [8] * (npk // 8)
        if npk % 8:
            spans.append(npk % 8)
        return spans

    # tanh offload: for every 5th group, ACT handles only packs 0-1 (FD
    # 1024) and the DVE computes packs 2-3 with the clamped poly chain in
    # fp16 fast modes (tensor_scalar 4x, tensor_tensor 2x). The conversion
    # fp32 PSUM -> fp16 SBUF runs in the same slot as the group's ACT half
    # (so the PSUM tile turns over no later than usual); the 6-op poly runs
    # as two 512-col pieces paced one per following group slot, and the
    # second-half output DMA is emitted after the last piece.
    poly_pool = ctx.enter_context(tc.tile_pool(name="poly", bufs=8))
    dve_q = []

    def mk_piece(s, t, A_, B_, ob):
        def emit():
            nc.vector.tensor_tensor(t[:], s[:], s[:], op=mult)
            nc.vector.tensor_scalar(A_[:], t[:], C5A, C5B, op0=mult, op1=add)
            nc.vector.tensor_tensor(B_[:], A_[:], t[:], op=mult)
            nc.vector.tensor_scalar(A_[:], B_[:], C5C, None, op0=add)
            nc.vector.tensor_tensor(B_[:], A_[:], s[:], op=mult)
            nc.vector.tensor_scalar(ob[:, 0:512], B_[:], CLAMP, -CLAMP,
                                    op0=amin, op1=amax)
        return emit

    ocol = 0
    nwarm = 0
    gg = 0
    for si, npk in enumerate(NPKS):
        P = P_tile
        spans = tr_spans(si, npk)
        x32 = None
        xbase = xnext = 0
        gi = 0
        last_super = si == len(NPKS) - 1
        for blk in range(0, npk, 4):
            gn = min(4, npk - blk)
            dstep = 2 if (last_super and blk >= 16) else 4
            ob = ob_pool.tile([128, 2048], F16)
            mm = mm_pool.tile([128, 2048], FP, tag="mmgrp")
            if blk >= xnext:
                if si > 0 and blk == 0:
                    # Build super si in one 4x copy, emitted right before
                    # its first transpose: late emission keeps its input-DMA
                    # wait out of the coalesced waits of earlier DVE
                    # batches (which would stall them until this super's
                    # input lands).
                    build(si, 0, npk)
                span = spans.pop(0)
                x32 = x_pool.tile([128, 1024], F16)
                xbase = xnext
                xnext = xbase + span
                nc.vector.transpose(x32[:, :span * 128],
                                    P[:, xbase * 128:xnext * 128])
            if si == 0 and gi < 3:
                for _ in range(2):
                    warm(mm, nwarm)
                    nwarm += 1
            for h in range(gn):
                hx = blk - xbase + h
                nc.tensor.matmul(mm[:, h * 512:(h + 1) * 512],
                                 x32[:, hx * 128:(hx + 1) * 128],
                                 wbig_sb[:], start=True, stop=True)
            offload = 1 <= si <= 6 and gi in (1, 3, 5)
            if offload:
                # ACT handles packs 1-3 (FD 1536, so the PSUM tile turns
                # over on the usual cadence); DVE computes pack 0. The conv
                # runs in this slot (ends before the 1536-col ACT does),
                # the 6-op poly piece one slot later, the DMA after that.
                nc.scalar.activation(ob[:, 512:2048], mm[:, 512:2048],
                                     mybir.ActivationFunctionType.Tanh)
                s = poly_pool.tile([128, 512], F16)
                t = poly_pool.tile([128, 512], F16)
                A_ = poly_pool.tile([128, 512], F16)
                B_ = poly_pool.tile([128, 512], F16)
                nc.vector.tensor_copy(s[:], mm[:, 0:512])
                oc2 = ocol + blk * 512

                def mk_dma(ob=ob, oc2=oc2):
                    def emit():
                        nc.sync.dma_start(out[:, oc2:oc2 + 512],
                                          ob[:, 0:512])
                    return emit

                dve_q += [mk_piece(s, t, A_, B_, ob), mk_dma()]
            else:
                nc.scalar.activation(ob[:, :gn * 512], mm[:, :gn * 512],
                                     mybir.ActivationFunctionType.Tanh)
            if pend:
                for job in pend.pop(0):
                    job()
            if dve_q:
                dve_q.pop(0)()
            if si == 0 and gi == 0:
                load_inputs(1)
            if blk == 4 and si + 2 < len(NPKS):
                load_inputs(si + 2)
            if offload:
                oc = ocol + blk * 512 + 512
                nc.sync.dma_start(out[:, oc:oc + 1536], ob[:, 512:2048])
            else:
                # The last two groups drain through the otherwise-idle
                # Scalar SEQ so their issues don't queue behind the Sync
                # queue's output backlog at the very end.
                eng = nc.scalar if (last_super and blk >= 16) else nc.sync
                for d0 in range(0, gn, dstep):
                    dn = min(dstep, gn - d0)
                    oc = ocol + (blk + d0) * 512
                    eng.dma_start(out[:, oc:oc + dn * 512],
                                  ob[:, d0 * 512:(d0 + dn) * 512])
            gi += 1
            gg += 1
        ocol += npk * 512
    while dve_q:
        dve_q.pop(0)()


def build_nc():
    if "nc" in _NC_CACHE:
        return _NC_CACHE["nc"]
    nc = bacc.Bacc("TRN2", target_bir_lowering=False, debug=False)
    icols = sum(npk * 36 for npk in NPKS)
    ocols = sum(npk * 512 for npk in NPKS)
    inc = nc.dram_tensor("inc", [128, icols], F16, kind="ExternalInput").ap()
    wbig = nc.dram_tensor("wbig", [128, 512], F16, kind="ExternalInput").ap()
    out = nc.dram_tensor("out", [128, ocols], F16, kind="ExternalOutput").ap()
    with tile.TileContext(nc) as tc:
        with ExitStack() as ctx:
            _body(ctx, tc, out, inc, wbig)
    nc.compile()
    _NC_CACHE["nc"] = nc
    return nc


def make_wbig(W_slack, b_slack, W_gen, b_gen, W_load, b_load):
    W_list = [np.asarray(w, np.float32) for w in (W_slack, W_gen, W_load)]
    b_list = [np.asarray(b, np.float32) for b in (b_slack, b_gen, b_load)]
    WBig = np.zeros((128, 512), np.float16)
    for A in range(4):
        col = A * 128
        for e in range(3):
            WBig[32 * A + 3 * e + 0, col:col + 128] = \
                W_list[e][0].astype(np.float16)
            WBig[32 * A + 3 * e + 1, col:col + 128] = \
                W_list[e][1].astype(np.float16)
            WBig[32 * A + 3 * e + 2, col:col + 128] = \
                b_list[e].astype(np.float16)
    return WBig


def _permute_inputs(featp, btp):
    """featp (npad, 2) f32, btp (npad,) f32 -> ind (8, 128, sum(npk*36)) f16:
    per pack pt, 36 cols holding the 9 pre-masked slot values
    (bus_type==e+1)*[f0, f1, 1] for each of 4 groups, in the device layout
    (p, pt, g, s). On-device build is then a single 4x-mode copy."""
    featp = featp.reshape(N_CORES, PER_CORE, 2)
    btp = btp.reshape(N_CORES, PER_CORE)
    parts = []
    off = 0
    for ssz, npk in zip(SUPERS, NPKS):
        f4 = featp[:, off:off + ssz].reshape(N_CORES, 4, 32, npk, 4, 2)
        # orig (c, B, j, pt, A, k) -> device (c, p=32A+j, pt, g=B, k)
        dv = f4.transpose(0, 4, 2, 3, 1, 5).reshape(
            N_CORES, 128, npk, 4, 2).astype(np.float16)
        b4 = btp[:, off:off + ssz].reshape(N_CORES, 4, 32, npk, 4)
        db = b4.transpose(0, 4, 2, 3, 1).reshape(N_CORES, 128, npk, 4)
        fx3 = np.empty((N_CORES, 128, npk, 4, 3), np.float16)
        fx3[..., :2] = dv
        fx3[..., 2] = 1.0
        in9 = np.empty((N_CORES, 128, npk, 4, 9), np.float16)
        for e in range(3):
            in9[..., 3 * e:3 * e + 3] = (db == e + 1)[..., None] * fx3
        parts.append(in9.reshape(N_CORES, 128, npk * 36))
        off += ssz
    return np.ascontiguousarray(np.concatenate(parts, axis=2))


def kernel(feat, bus_type, W_slack, b_slack, W_gen, b_gen, W_load, b_load,
           **run_kwargs):
    feat = np.asarray(feat, np.float32)
    bt = np.asarray(bus_type)
    n = feat.shape[0]
    npad = N_CORES * PER_CORE
    assert n <= npad

    featp = np.zeros((npad, 2), np.float32)
    featp[:n] = feat
    btp = np.zeros(npad, np.float32)
    btp[:n] = bt.astype(np.float32)
    ind = _permute_inputs(featp, btp)
    wbig = make_wbig(W_slack, b_slack, W_gen, b_gen, W_load, b_load)

    nc = build_nc()
    in_maps = [
        {"inc": ind[i], "wbig": wbig}
        for i in range(N_CORES)
    ]
    try:
        res = run_bass_kernel_spmd(nc, in_maps, list(range(N_CORES)),
                                   **run_kwargs)
    except Exception:
        # A previously-failed process can leave the NeuronCores wedged
        # (NRT_EXEC_UNIT_UNRECOVERABLE); a small probe op resets them.
        import time as _time

        import jax.numpy as jnp

        for _ in range(3):
            try:
                float(jnp.sum(jnp.ones((8, 8))))
                break
            except Exception:
                _time.sleep(5)
        res = run_bass_kernel_spmd(nc, in_maps, list(range(N_CORES)),
                                   **run_kwargs)

    outs = []
    for i in range(N_CORES):
        dev = res.results[i]["out"]  # (128, 125952) f16
        off = 0
        parts = []
        for ssz, npk in zip(SUPERS, NPKS):
            block = dev[:, off:off + npk * 512].reshape(128, npk, 4, 128)
            parts.append(block.reshape(ssz, D))
            off += npk * 512
        outs.append(np.concatenate(parts, axis=0))
    out = np.concatenate(outs, axis=0)
    kernel.last_result = res
    return out[:n].astype(np.float32)



# revision 47
# speedup vs baseline: 1.0072x; 1.0072x over previous
"""fp16-output MoE routing kernel (tanh(feat @ W_e + b_e) routed by bus_type).

The correctness gate is rel_err < 2e-2 on fixed-seed inputs. ScalarE tanh
is the primary bottleneck (1 elem/cycle/partition @ 1.2GHz over 125,952
cols/core = 105us + per-ACTIVATE overhead), so part of the tanh work is
offloaded to the DVE as a clamped degree-5 odd minimax polynomial
(end-to-end max err ~1.5e-2, deterministic). DMA is the secondary wall
(output 32.2MB/core at ~380GB/s aggregate).

Data flow per 512-token pack:
- Host pre-masks inputs: 9 f16 slot values/token (bus_type==e+1)*[f0,f1,1]
  in device layout, so the on-device "build" is ONE DVE tensor_copy per
  supertile running in the 4x perf mode (tensor_scalar/tensor_copy support
  4x on fp16 SBUF; scalar_tensor_tensor has NO fast mode — the old
  mask-on-device build was 6x more DVE time).
- P[p, 32g+s] slot matrix (single persistent tile): slots 9..31 zeroed
  once via 4x copies from a zeroed dummy tile (NOT GpSimd memset: GpSimd
  shares an SBUF port with the DVE and a concurrent big memset slows DVE
  4x copies ~20x; its completion sem also lands in coalesced DVE waits).
- DVE stream-transpose (32x32 blocks) per 8 packs -> X32.
- matmul(out, lhsT=X32 slice, rhs=Wbig) with Wbig block-diagonal over
  A-blocks: out[32B+j, A*128+d] = z(token) in PSUM fp32 (TRN2 matmul
  cannot write fp16 PSUM).
- ACT tanh FD 2048 -> fp16 SBUF ob tile; for offloaded groups (gi 1,3
  of middle supertiles) ACT does packs 1-3 (FD 1536, which keeps the PSUM
  tile turnover on the normal cadence and stays above the ~1.5us/group PE
  refill floor) while the DVE converts pack 0 (fp32 PSUM -> fp16 SBUF,
  1x) in the same slot and runs the 6-op poly piece one slot later:
  t=s*s (TT 2x); u=C5A*t+C5B (TS 4x); h=u*t; g=h+C5C; y=g*s;
  clamp(+-CLAMP) into ob. The pack-0 output DMA is deferred two slots.
- Output [128, npk*512] per supertile; host permutation keeps HBM lines
  contiguous and the unpack a pure reshape.

Scheduling notes (hard-won):
- Input loads are issued from the Scalar SEQ (667ns issue slots hide
  between TANH dispatches), one merged DMA per supertile, staggered 2
  supertiles ahead; supertile 0 is split 8+24 packs so the first build
  only waits on the 74KB head. The Sync queue stays output-only.
- The Tile scheduler coalesces upcoming DVE waits into one
  EVENT_SEMAPHORE (window spans ~a dozen instructions), so each
  supertile's build is emitted immediately before its first transpose:
  emitting it earlier stalls the whole early DVE batch on that
  supertile's input DMA.
- PE warmup matmuls (2 per group over the first 3 groups, into the
  group's own mm tile just before the real matmuls overwrite it)
  un-throttle the HAM clock gate; ACT gaps let it re-throttle, which
  shows up as slow MATMULs.
"""

import sys
from contextlib import ExitStack

import numpy as np

sys.path.insert(0, "/opt/trn_rl_repo")

import concourse.bacc as bacc  # noqa: E402
import concourse.mybir as mybir  # noqa: E402
import concourse.tile as tile  # noqa: E402
from concourse.bass_utils import run_bass_kernel_spmd  # noqa: E402

FP = mybir.dt.float32
F16 = mybir.dt.float16
D = 128
PACK = 512
SUPERS = [16384] * 7 + [11264]
NPKS = [s // PACK for s in SUPERS]
N_CORES = 8
PER_CORE = sum(SUPERS)

_NC_CACHE = {}


# Clamped odd minimax polynomial for tanh on the DVE-offloaded packs:
# y = clip(x*(C5A*x^4 + C5B*x^2 + C5C), +-CLAMP). LP-fit over z range
# [-4.7, 4.7]; poly error 0.0140, ~0.0155 end to end with the fp16 chain
# (gate is 2e-2, deterministic inputs).
C5A = 0.019074705123782443
C5B = -0.1933162963179165
C5C = 0.9452190991740973
CLAMP = 1.0 - 0.01273


def _body(ctx, tc, out, inc, wbig):
    nc = tc.nc
    mult = mybir.AluOpType.mult
    add = mybir.AluOpType.add
    amin = mybir.AluOpType.min
    amax = mybir.AluOpType.max

    const_pool = ctx.enter_context(tc.tile_pool(name="const", bufs=1))
    # ONE persistent slot matrix: builds are cheap 4x copies now, so super
    # si+1's build simply runs after super si's last transpose (in-order
    # DVE). Slots 9..31 are zeroed once and never written again (their Wbig
    # rows are zero, but NaN garbage would still poison the accumulation).
    # All zeroing stays on the DVE: GpSimd memsets contend with DVE for the
    # shared SBUF port (a concurrent 4x DVE copy was observed to crawl
    # ~20x) and their completion semaphore stalls the first builds. The
    # P tail is zeroed by 4x tensor_copies from the zeroed dummy tile,
    # paced through the first group slots below.
    P_tile = const_pool.tile([128, 32 * 128], F16, name="Pbuf")
    # Zeroed dummy operand for PE warmup (and zero-source for P), then the
    # first 4 packs of P.
    dumw = const_pool.tile([128, 512], F16)
    nc.vector.memset(dumw[:], 0.0)
    nc.vector.memset(P_tile[:, :4 * 128], 0.0)

    in_pool = ctx.enter_context(tc.tile_pool(name="inp", bufs=1))
    x_pool = ctx.enter_context(tc.tile_pool(name="x32", bufs=3))
    mm_pool = ctx.enter_context(tc.tile_pool(name="mm", bufs=2, space="PSUM"))
    ob_pool = ctx.enter_context(tc.tile_pool(name="ob", bufs=8))

    # Per-supertile input tiles: the 9 pre-masked slot values per token (36
    # f16 cols/pack), one dma_start per supertile with 2.25KB-contiguous
    # partition rows. All input loads are issued from the Scalar SEQ: it
    # only dispatches one TANH every ~1.85us, so the ~0.7us issue slots
    # hide there, keeping the Sync queue output-only.
    in_tiles = []
    icol = 0
    for si, npk in enumerate(NPKS):
        inT = in_pool.tile([128, npk * 36], F16, name=f"inT{si}")
        in_tiles.append((inT, icol))
        icol += npk * 36

    def load_inputs(si):
        inT, ic = in_tiles[si]
        npk = NPKS[si]
        nc.scalar.dma_start(inT[:], inc[:, ic:ic + npk * 36])

    # Supertile 0's load goes out first (before the ACT table load), split
    # so the first 8 packs (74KB) land ~2.5us before the rest: the first
    # build is the critical path to the first TANH. Supertile 1 is loaded
    # from inside the loop.
    inT0, _ = in_tiles[0]
    nc.scalar.dma_start(inT0[:, :8 * 36], inc[:, :8 * 36])
    nc.scalar.dma_start(inT0[:, 8 * 36:], inc[:, 8 * 36:NPKS[0] * 36])
    wbig_sb = const_pool.tile([128, 512], F16)
    nc.sync.dma_start(wbig_sb[:], wbig)

    # PE warmup (HAM un-throttle needs ~3.4us of busy) runs on the zeroed
    # dummy operand. Warmups write into the first groups' own mm tiles just
    # before the real matmuls overwrite them (start=True resets PSUM), so
    # no separate PSUM tile is needed and nothing queues behind them.
    def warm(mm, w):
        nc.tensor.matmul(mm[:, (w % 4) * 512:(w % 4 + 1) * 512],
                         dumw[:, :128], dumw[:], start=True, stop=True)

    # Builds run on the DVE, split into chunks small enough to fit the
    # per-group slack between stream-transposes (ACT pace ~1.85us/group,
    # transposes ~0.6us/group).
    def build(si, lo, hi):
        inT, _ = in_tiles[si]
        P4 = P_tile[:, lo * 128:hi * 128].rearrange(
            "p (pt g s) -> p pt g s", g=4, s=32)
        in9 = inT[:, lo * 36:hi * 36].rearrange(
            "p (pt g s) -> p pt g s", g=4, s=9)
        nc.vector.tensor_copy(P4[:, :, :, 0:9], in9)

    build(0, 0, 4)
    # One-time zero fill of P packs 4..32 (4x copies of the zeroed dummy
    # into all 32 slots) ordered BEFORE the build of the same pack range;
    # paced one job list per group slot during super 0's ramp. Slots 9..31
    # are never written again; later builds only rewrite slots 0..8.
    def pzero(lo, hi):
        for c in range(lo * 128, hi * 128, 512):
            nc.vector.tensor_copy(P_tile[:, c:c + 512], dumw[:])

    pzero(4, 8)
    pend = [
        [lambda: build(0, 4, 8)],
        [lambda: pzero(8, 16), lambda: build(0, 8, 16)],
        [lambda: pzero(16, 24), lambda: build(0, 16, 24)],
        [lambda: pzero(24, 32), lambda: build(0, 24, 32)],
    ]

    # Transpose spans per supertile: 8-pack transposes halve the DVE
    # per-op overhead; supertile 0 starts with two 4-pack ones so the first
    # TANH isn't gated on an 8-pack build+transpose.
    def tr_spans(si, npk):
        if si == 0:
            return [4, 4] + [8] * ((npk - 8) // 8)
        spans = [8] * (npk // 8)
        if npk % 8:
            spans.append(npk % 8)
        return spans

    # tanh offload: for every 5th group, ACT handles only packs 0-1 (FD
    # 1024) and the DVE computes packs 2-3 with the clamped poly chain in
    # fp16 fast modes (tensor_scalar 4x, tensor_tensor 2x). The conversion
    # fp32 PSUM -> fp16 SBUF runs in the same slot as the group's ACT half
    # (so the PSUM tile turns over no later than usual); the 6-op poly runs
    # as two 512-col pieces paced one per following group slot, and the
    # second-half output DMA is emitted after the last piece.
    poly_pool = ctx.enter_context(tc.tile_pool(name="poly", bufs=8))
    dve_q = []

    def mk_piece(s, t, A_, B_, ob):
        def emit():
            nc.vector.tensor_tensor(t[:], s[:], s[:], op=mult)
            nc.vector.tensor_scalar(A_[:], t[:], C5A, C5B, op0=mult, op1=add)
            nc.vector.tensor_tensor(B_[:], A_[:], t[:], op=mult)
            nc.vector.tensor_scalar(A_[:], B_[:], C5C, None, op0=add)
            nc.vector.tensor_tensor(B_[:], A_[:], s[:], op=mult)
            nc.vector.tensor_scalar(ob[:, 0:512], B_[:], CLAMP, -CLAMP,
                                    op0=amin, op1=amax)
        return emit

    ocol = 0
    nwarm = 0
    gg = 0
    for si, npk in enumerate(NPKS):
        P = P_tile
        spans = tr_spans(si, npk)
        x32 = None
        xbase = xnext = 0
        gi = 0
        last_super = si == len(NPKS) - 1
        for blk in range(0, npk, 4):
            gn = min(4, npk - blk)
            dstep = 2 if (last_super and blk >= 16) else 4
            ob = ob_pool.tile([128, 2048], F16)
            mm = mm_pool.tile([128, 2048], FP, tag="mmgrp")
            if blk >= xnext:
                if si > 0 and blk == 0:
                    # Build super si in one 4x copy, emitted right before
                    # its first transpose: late emission keeps its input-DMA
                    # wait out of the coalesced waits of earlier DVE
                    # batches (which would stall them until this super's
                    # input lands).
                    build(si, 0, npk)
                span = spans.pop(0)
                x32 = x_pool.tile([128, 1024], F16)
                xbase = xnext
                xnext = xbase + span
                nc.vector.transpose(x32[:, :span * 128],
                                    P[:, xbase * 128:xnext * 128])
            if si == 0 and gi < 3:
                for _ in range(2):
                    warm(mm, nwarm)
                    nwarm += 1
            for h in range(gn):
                hx = blk - xbase + h
                nc.tensor.matmul(mm[:, h * 512:(h + 1) * 512],
                                 x32[:, hx * 128:(hx + 1) * 128],
                                 wbig_sb[:], start=True, stop=True)
            offload = 1 <= si <= 6 and gi in (1, 3)
            if offload:
                # ACT handles packs 1-3 (FD 1536, so the PSUM tile turns
                # over on the usual cadence); DVE computes pack 0. The conv
                # runs in this slot (ends before the 1536-col ACT does),
                # the 6-op poly piece one slot later, the DMA after that.
                nc.scalar.activation(ob[:, 512:2048], mm[:, 512:2048],
                                     mybir.ActivationFunctionType.Tanh)
                s = poly_pool.tile([128, 512], F16)
                t = poly_pool.tile([128, 512], F16)
                A_ = poly_pool.tile([128, 512], F16)
                B_ = poly_pool.tile([128, 512], F16)
                nc.vector.tensor_copy(s[:], mm[:, 0:512])
                oc2 = ocol + blk * 512

                def mk_dma(ob=ob, oc2=oc2):
                    def emit():
                        nc.sync.dma_start(out[:, oc2:oc2 + 512],
                                          ob[:, 0:512])
                    return emit

                dve_q += [mk_piece(s, t, A_, B_, ob), mk_dma()]
            else:
                nc.scalar.activation(ob[:, :gn * 512], mm[:, :gn * 512],
                                     mybir.ActivationFunctionType.Tanh)
            if pend:
                for job in pend.pop(0):
                    job()
            if dve_q:
                dve_q.pop(0)()
            if si == 0 and gi == 0:
                load_inputs(1)
            if blk == 4 and si + 2 < len(NPKS):
                load_inputs(si + 2)
            if offload:
                oc = ocol + blk * 512 + 512
                nc.sync.dma_start(out[:, oc:oc + 1536], ob[:, 512:2048])
            else:
                for d0 in range(0, gn, dstep):
                    dn = min(dstep, gn - d0)
                    oc = ocol + (blk + d0) * 512
                    nc.sync.dma_start(out[:, oc:oc + dn * 512],
                                      ob[:, d0 * 512:(d0 + dn) * 512])
            gi += 1
            gg += 1
        ocol += npk * 512
    while dve_q:
        dve_q.pop(0)()


def build_nc():
    if "nc" in _NC_CACHE:
        return _NC_CACHE["nc"]
    nc = bacc.Bacc("TRN2", target_bir_lowering=False, debug=False)
    icols = sum(npk * 36 for npk in NPKS)
    ocols = sum(npk * 512 for npk in NPKS)
    inc = nc.dram_tensor("inc", [128, icols], F16, kind="ExternalInput").ap()
    wbig = nc.dram_tensor("wbig", [128, 512], F16, kind="ExternalInput").ap()
    out = nc.dram_tensor("out", [128, ocols], F16, kind="ExternalOutput").ap()
    with tile.TileContext(nc) as tc:
        with ExitStack() as ctx:
            _body(ctx, tc, out, inc, wbig)
    nc.compile()
    _NC_CACHE["nc"] = nc
    return nc


def make_wbig(W_slack, b_slack, W_gen, b_gen, W_load, b_load):
    W_list = [np.asarray(w, np.float32) for w in (W_slack, W_gen, W_load)]
    b_list = [np.asarray(b, np.float32) for b in (b_slack, b_gen, b_load)]
    WBig = np.zeros((128, 512), np.float16)
    for A in range(4):
        col = A * 128
        for e in range(3):
            WBig[32 * A + 3 * e + 0, col:col + 128] = \
                W_list[e][0].astype(np.float16)
            WBig[32 * A + 3 * e + 1, col:col + 128] = \
                W_list[e][1].astype(np.float16)
            WBig[32 * A + 3 * e + 2, col:col + 128] = \
                b_list[e].astype(np.float16)
    return WBig


def _permute_inputs(featp, btp):
    """featp (npad, 2) f32, btp (npad,) f32 -> ind (8, 128, sum(npk*36)) f16:
    per pack pt, 36 cols holding the 9 pre-masked slot values
    (bus_type==e+1)*[f0, f1, 1] for each of 4 groups, in the device layout
    (p, pt, g, s). On-device build is then a single 4x-mode copy."""
    featp = featp.reshape(N_CORES, PER_CORE, 2)
    btp = btp.reshape(N_CORES, PER_CORE)
    parts = []
    off = 0
    for ssz, npk in zip(SUPERS, NPKS):
        f4 = featp[:, off:off + ssz].reshape(N_CORES, 4, 32, npk, 4, 2)
        # orig (c, B, j, pt, A, k) -> device (c, p=32A+j, pt, g=B, k)
        dv = f4.transpose(0, 4, 2, 3, 1, 5).reshape(
            N_CORES, 128, npk, 4, 2).astype(np.float16)
        b4 = btp[:, off:off + ssz].reshape(N_CORES, 4, 32, npk, 4)
        db = b4.transpose(0, 4, 2, 3, 1).reshape(N_CORES, 128, npk, 4)
        fx3 = np.empty((N_CORES, 128, npk, 4, 3), np.float16)
        fx3[..., :2] = dv
        fx3[..., 2] = 1.0
        in9 = np.empty((N_CORES, 128, npk, 4, 9), np.float16)
        for e in range(3):
            in9[..., 3 * e:3 * e + 3] = (db == e + 1)[..., None] * fx3
        parts.append(in9.reshape(N_CORES, 128, npk * 36))
        off += ssz
    return np.ascontiguousarray(np.concatenate(parts, axis=2))


def kernel(feat, bus_type, W_slack, b_slack, W_gen, b_gen, W_load, b_load,
           **run_kwargs):
    feat = np.asarray(feat, np.float32)
    bt = np.asarray(bus_type)
    n = feat.shape[0]
    npad = N_CORES * PER_CORE
    assert n <= npad

    featp = np.zeros((npad, 2), np.float32)
    featp[:n] = feat
    btp = np.zeros(npad, np.float32)
    btp[:n] = bt.astype(np.float32)
    ind = _permute_inputs(featp, btp)
    wbig = make_wbig(W_slack, b_slack, W_gen, b_gen, W_load, b_load)

    nc = build_nc()
    in_maps = [
        {"inc": ind[i], "wbig": wbig}
        for i in range(N_CORES)
    ]
    try:
        res = run_bass_kernel_spmd(nc, in_maps, list(range(N_CORES)),
                                   **run_kwargs)
    except Exception:
        # A previously-failed process can leave the NeuronCores wedged
        # (NRT_EXEC_UNIT_UNRECOVERABLE); a small probe op resets them.
        import time as _time

        import jax.numpy as jnp

        for _ in range(3):
            try:
                float(jnp.sum(jnp.ones((8, 8))))
                break
            except Exception:
                _time.sleep(5)
        res = run_bass_kernel_spmd(nc, in_maps, list(range(N_CORES)),
                                   **run_kwargs)

    outs = []
    for i in range(N_CORES):
        dev = res.results[i]["out"]  # (128, 125952) f16
        off = 0
        parts = []
        for ssz, npk in zip(SUPERS, NPKS):
            block = dev[:, off:off + npk * 512].reshape(128, npk, 4, 128)
            parts.append(block.reshape(ssz, D))
            off += npk * 512
        outs.append(np.concatenate(parts, axis=0))
    out = np.concatenate(outs, axis=0)
    kernel.last_result = res
    return out[:n].astype(np.float32)

